# revision 10
# baseline (speedup 1.0000x reference)
"""GCNFast Trainium2 kernel.

out[b] = relu(A @ x_b + GCB),  A = relu(AA_mask * GCW)  [4096, 4096]
x_b = transpose(h[b]) reshaped [Nt*Nc, d_h];  out reshaped to [bs, Ns, Nt, d_h].

Sharding over 8 cores: 4-way row-shard of A/GCB (1024 rows each) x 2-way
batch split (8 batches each). Each core computes its [1024, 4096] slice of A
on-chip (DVE mul -> PE transpose to k-major -> ACT relu), keeps the bf16
activations X [4096, 8*128] resident in SBUF, and accumulates 64 bf16
matmuls per m-tile into PSUM, with a DVE bias-add + ACT relu epilogue.

Index conventions inside a core (both are pure permutations absorbed by the
on-chip transpose stage, chosen so every DMA access pattern collapses to
<=3 dims with a contiguous partition merge):
 - contraction k' = c*Nt + t  (c-major), so h's (c t) merges contiguously;
 - output row m' = s*Tsh + t  (s-major), so out's (s t) merges contiguously.
"""

from contextlib import ExitStack

import numpy as np

import concourse.mybir as mybir
import concourse.tile as tile
from concourse import bacc, masks
from concourse.bass_utils import run_bass_kernel_spmd

# Problem constants (hardcoded per harness contract).
NC_, NS, NT, DH, BS = 64, 64, 64, 128, 16
K = NC_ * NT          # 4096 contraction dim
M = NS * NT           # 4096 output rows
P_ROW, P_BATCH = 4, 2  # 4-way row shard x 2-way batch shard = 8 cores
M_SH = M // P_ROW     # 1024 rows per core
B_SH = BS // P_BATCH  # 8 batches per core
NFREE = B_SH * DH     # 1024 = moving free dim (b, d)
KT = K // 128         # 32 k-tiles
MT = M_SH // 128      # 8 m-tiles per core
T_SH = M_SH // NS     # 16 t-values per core
S_PT = 128 // T_SH    # 8 s-values per m'-tile

F32 = mybir.dt.float32
BF16 = mybir.dt.bfloat16

_cached = {}


def _build():
    nc = bacc.Bacc(
        "TRN2",
        target_bir_lowering=False,
        debug=False,
        enable_asserts=False,
        num_devices=8,
    )

    gcw = nc.dram_tensor("gcw", [M_SH, K], F32, kind="ExternalInput").ap()
    aa = nc.dram_tensor("aa", [M_SH, K], F32, kind="ExternalInput").ap()
    gcb = nc.dram_tensor("gcb", [M_SH, DH], F32, kind="ExternalInput").ap()
    h = nc.dram_tensor("h", [B_SH, NC_, NT, DH], F32, kind="ExternalInput").ap()
    out = nc.dram_tensor("out", [B_SH, NS, T_SH, DH], F32, kind="ExternalOutput").ap()

    # row-permuted views: m' = s*T_SH + t  (s-major)
    gcw_p = gcw.rearrange("(t s) k -> s t k", t=T_SH)
    aa_p = aa.rearrange("(t s) k -> s t k", t=T_SH)
    gcb_p = gcb.rearrange("(t s) d -> s t d", t=T_SH)

    with tile.TileContext(nc) as tc:
        with ExitStack() as ctx:
            ident_pool = ctx.enter_context(tc.tile_pool(name="ident", bufs=1))
            x_pool = ctx.enter_context(tc.tile_pool(name="x", bufs=KT))
            gw_pool = ctx.enter_context(tc.tile_pool(name="gw", bufs=4))
            aa_pool = ctx.enter_context(tc.tile_pool(name="aam", bufs=4))
            am_pool = ctx.enter_context(tc.tile_pool(name="am", bufs=2))
            at_pool = ctx.enter_context(tc.tile_pool(name="at", bufs=2))
            gcb_pool = ctx.enter_context(tc.tile_pool(name="gcb", bufs=MT))
            out_pool = ctx.enter_context(tc.tile_pool(name="out", bufs=2))
            ptr_pool = ctx.enter_context(
                tc.tile_pool(name="ptr", bufs=2, space="PSUM")
            )
            pmm_pool = ctx.enter_context(
                tc.tile_pool(name="pmm", bufs=2, space="PSUM")
            )

            ident = ident_pool.tile([128, 128], BF16)
            masks.make_identity(nc, ident[:])

            # Interleave the A-stream prefetch (per-m-tile critical path
            # feeder) with the resident X tiles so neither starves: queue
            # order on the SWDGE ring follows program order.
            gw_tiles, aa_tiles, gcb_tiles, x_tiles = [], [], [], []
            for mt in range(MT):
                srows = slice(S_PT * mt, S_PT * (mt + 1))
                gw_t = gw_pool.tile([128, K], BF16)
                nc.gpsimd.dma_start(out=gw_t[:], in_=gcw_p[srows])
                aa_t = aa_pool.tile([128, K], BF16)
                nc.gpsimd.dma_start(out=aa_t[:], in_=aa_p[srows])
                gw_tiles.append(gw_t)
                aa_tiles.append(aa_t)
                # X[k'-tile] = [128 (c,t), 1024 (b,d)], cast f32->bf16 in
                # the SWDGE DMA datapath; 4 per m-tile covers all 32.
                for j in range(4 * mt, 4 * mt + 4):
                    xt = x_pool.tile([128, NFREE], BF16)
                    src = h[:, 2 * j : 2 * j + 2, :, :].rearrange(
                        "b c t d -> (c t) b d"
                    )
                    nc.gpsimd.dma_start(out=xt[:], in_=src)
                    x_tiles.append(xt)
                if mt == 0:
                    for mt2 in range(MT):
                        srows2 = slice(S_PT * mt2, S_PT * (mt2 + 1))
                        gcb_t = gcb_pool.tile([128, DH], F32)
                        nc.sync.dma_start(out=gcb_t[:], in_=gcb_p[srows2])
                        gcb_tiles.append(gcb_t)

            for mt in range(MT):
                gw_t, aa_t = gw_tiles[mt], aa_tiles[mt]
                # masked weights with fused relu: since aa >= 0,
                # relu(gw*aa) == max(gw,0)*aa. The output AP permutes the
                # free dim from t-major k to c-major k' so the transpose and
                # matmul reads stay dense:
                # am_t[m, c*Nt + t] = max(gw[m, t*Nc+c], 0) * aa[m, t*Nc+c].
                am_t = am_pool.tile([128, K], BF16)
                nc.vector.scalar_tensor_tensor(
                    am_t[:].rearrange("m (c t) -> m t c", c=NC_),
                    gw_t[:].rearrange("m (t c) -> m t c", c=NC_),
                    0.0,
                    aa_t[:].rearrange("m (t c) -> m t c", c=NC_),
                    mybir.AluOpType.max,
                    mybir.AluOpType.mult,
                )

                # A^T for this m'-tile: 32 side-by-side [128 k', 128 m'] tiles.
                at_t = at_pool.tile([128, K], BF16)
                for g in range(KT // 8):
                    ptr = ptr_pool.tile([128, 1024], BF16)
                    for j8 in range(8):
                        j = 8 * g + j8
                        nc.tensor.transpose(
                            ptr[:, 128 * j8 : 128 * j8 + 128],
                            am_t[:, 128 * j : 128 * j + 128],
                            ident[:],
                        )
                    dstslice = at_t[:, 1024 * g : 1024 * g + 1024]
                    if g % 2 == 0:
                        nc.scalar.copy(dstslice, ptr[:])
                    else:
                        nc.vector.tensor_copy(dstslice, ptr[:])

                # 32 accumulating matmuls: psum[m', (b,d)] += A^T[k']^T @ X[k']
                pm = pmm_pool.tile([128, NFREE], F32)
                for j in range(KT):
                    for nh in range(NFREE // 512):
                        nc.tensor.matmul(
                            pm[:, 512 * nh : 512 * nh + 512],
                            at_t[:, 128 * j : 128 * j + 128],
                            x_tiles[j][:, 512 * nh : 512 * nh + 512],
                            start=(j == 0),
                            stop=(j == KT - 1),
                        )

                # epilogue: bias add (broadcast over b) + relu, then store
                o_t = out_pool.tile([128, NFREE], F32)
                bias = gcb_tiles[mt][:].unsqueeze(1).broadcast_to(
                    (128, B_SH, DH)
                )
                nc.vector.tensor_add(
                    o_t[:].rearrange("p (b d) -> p b d", b=B_SH),
                    pm[:].rearrange("p (b d) -> p b d", b=B_SH),
                    bias,
                )
                nc.scalar.activation(
                    o_t[:], o_t[:], mybir.ActivationFunctionType.Relu
                )

                srows = slice(S_PT * mt, S_PT * (mt + 1))
                dst = out[:, srows, :, :].rearrange("b s t d -> (s t) b d")
                nc.sync.dma_start(out=dst, in_=o_t[:])

    nc.compile()
    return nc


def _build_compact():
    """Variant for the (expected) tiled AA_mask: mask[m, k] depends only on
    (m % Ns, k % Nc), so each core loads a tiny per-m-tile [128, Nc] mask
    instead of the full 16.8MB shard — per-core HBM reads drop ~33%."""
    nc = bacc.Bacc(
        "TRN2",
        target_bir_lowering=False,
        debug=False,
        enable_asserts=False,
        num_devices=8,
    )

    gcw = nc.dram_tensor("gcw", [M_SH, K], F32, kind="ExternalInput").ap()
    msk = nc.dram_tensor("msk", [MT, 128, NC_], F32, kind="ExternalInput").ap()
    gcb = nc.dram_tensor("gcb", [M_SH, DH], F32, kind="ExternalInput").ap()
    h = nc.dram_tensor("h", [B_SH, NC_, NT, DH], F32, kind="ExternalInput").ap()
    out = nc.dram_tensor("out", [B_SH, NS, T_SH, DH], F32, kind="ExternalOutput").ap()

    gcw_p = gcw.rearrange("(t s) k -> s t k", t=T_SH)
    gcb_p = gcb.rearrange("(t s) d -> s t d", t=T_SH)

    with tile.TileContext(nc) as tc:
        with ExitStack() as ctx:
            ident_pool = ctx.enter_context(tc.tile_pool(name="ident", bufs=1))
            x_pool = ctx.enter_context(tc.tile_pool(name="x", bufs=KT))
            gw_pool = ctx.enter_context(tc.tile_pool(name="gw", bufs=6))
            msk_pool = ctx.enter_context(tc.tile_pool(name="msk", bufs=MT))
            am_pool = ctx.enter_context(tc.tile_pool(name="am", bufs=2))
            at_pool = ctx.enter_context(tc.tile_pool(name="at", bufs=2))
            gcb_pool = ctx.enter_context(tc.tile_pool(name="gcb", bufs=MT))
            out_pool = ctx.enter_context(tc.tile_pool(name="out", bufs=2))
            ptr_pool = ctx.enter_context(
                tc.tile_pool(name="ptr", bufs=2, space="PSUM")
            )
            pmm_pool = ctx.enter_context(
                tc.tile_pool(name="pmm", bufs=2, space="PSUM")
            )

            ident = ident_pool.tile([128, 128], BF16)
            masks.make_identity(nc, ident[:])

            gw_tiles, msk_tiles, gcb_tiles, x_tiles = [], [], [], []
            for mt in range(MT):
                srows = slice(S_PT * mt, S_PT * (mt + 1))
                gw_t = gw_pool.tile([128, K], BF16)
                nc.gpsimd.dma_start(out=gw_t[:], in_=gcw_p[srows])
                gw_tiles.append(gw_t)
                for j in range(4 * mt, 4 * mt + 4):
                    xt = x_pool.tile([128, NFREE], BF16)
                    src = h[:, 2 * j : 2 * j + 2, :, :].rearrange(
                        "b c t d -> (c t) b d"
                    )
                    nc.gpsimd.dma_start(out=xt[:], in_=src)
                    x_tiles.append(xt)
                if mt == 0:
                    for mt2 in range(MT):
                        m_t = msk_pool.tile([128, NC_], BF16)
                        nc.gpsimd.dma_start(out=m_t[:], in_=msk[mt2])
                        msk_tiles.append(m_t)
                        srows2 = slice(S_PT * mt2, S_PT * (mt2 + 1))
                        gcb_t = gcb_pool.tile([128, DH], F32)
                        nc.sync.dma_start(out=gcb_t[:], in_=gcb_p[srows2])
                        gcb_tiles.append(gcb_t)

            for mt in range(MT):
                # A = relu(gw) * mask (mask >= 0), permuted to c-major k',
                # with the mask broadcast along t (it only depends on (s, c)).
                am_t = am_pool.tile([128, K], BF16)
                nc.vector.scalar_tensor_tensor(
                    am_t[:].rearrange("m (c t) -> m t c", c=NC_),
                    gw_tiles[mt][:].rearrange("m (t c) -> m t c", c=NC_),
                    0.0,
                    msk_tiles[mt][:].unsqueeze(1).broadcast_to((128, NT, NC_)),
                    mybir.AluOpType.max,
                    mybir.AluOpType.mult,
                )

                at_t = at_pool.tile([128, K], BF16)
                for g in range(KT // 8):
                    ptr = ptr_pool.tile([128, 1024], BF16)
                    for j8 in range(8):
                        j = 8 * g + j8
                        nc.tensor.transpose(
                            ptr[:, 128 * j8 : 128 * j8 + 128],
                            am_t[:, 128 * j : 128 * j + 128],
                            ident[:],
                        )
                    dstslice = at_t[:, 1024 * g : 1024 * g + 1024]
                    if g % 2 == 0:
                        nc.scalar.copy(dstslice, ptr[:])
                    else:
                        nc.vector.tensor_copy(dstslice, ptr[:])

                pm = pmm_pool.tile([128, NFREE], F32)
                for j in range(KT):
                    for nh in range(NFREE // 512):
                        nc.tensor.matmul(
                            pm[:, 512 * nh : 512 * nh + 512],
                            at_t[:, 128 * j : 128 * j + 128],
                            x_tiles[j][:, 512 * nh : 512 * nh + 512],
                            start=(j == 0),
                            stop=(j == KT - 1),
                        )

                o_t = out_pool.tile([128, NFREE], F32)
                bias = gcb_tiles[mt][:].unsqueeze(1).broadcast_to(
                    (128, B_SH, DH)
                )
                nc.vector.tensor_add(
                    o_t[:].rearrange("p (b d) -> p b d", b=B_SH),
                    pm[:].rearrange("p (b d) -> p b d", b=B_SH),
                    bias,
                )
                nc.scalar.activation(
                    o_t[:], o_t[:], mybir.ActivationFunctionType.Relu
                )

                srows = slice(S_PT * mt, S_PT * (mt + 1))
                dst = out[:, srows, :, :].rearrange("b s t d -> (s t) b d")
                nc.sync.dma_start(out=dst, in_=o_t[:])

    nc.compile()
    return nc


def _mask_small(AA_mask):
    """[MT, 128, Nc] per-m'-tile mask rows (same for every core)."""
    A64 = AA_mask[:NS, :NC_]
    ms = np.empty((MT, 128, NC_), dtype=np.float32)
    for mt in range(MT):
        for p in range(128):
            s = S_PT * mt + p // T_SH
            ms[mt, p] = A64[s]
    return ms


def _is_tiled(AA_mask):
    A64 = AA_mask[:NS, :NC_]
    return np.array_equal(AA_mask, np.tile(A64, (NT, NT)))


def _make_in_maps(h, AA_mask, GCW, GCB, compact):
    in_maps = []
    ms = _mask_small(AA_mask) if compact else None
    for r in range(8):
        rq, bq = r % P_ROW, r // P_ROW
        rs = slice(M_SH * rq, M_SH * (rq + 1))
        bs_ = slice(B_SH * bq, B_SH * (bq + 1))
        m = {
            "gcw": np.ascontiguousarray(GCW[rs], np.float32),
            "gcb": np.ascontiguousarray(GCB[rs], np.float32),
            "h": np.ascontiguousarray(h[bs_], np.float32),
        }
        if compact:
            m["msk"] = ms
        else:
            m["aa"] = np.ascontiguousarray(AA_mask[rs], np.float32)
        in_maps.append(m)
    return in_maps


def _assemble(results):
    full = np.empty((BS, NS, NT, DH), dtype=np.float32)
    for r in range(8):
        rq, bq = r % P_ROW, r // P_ROW
        full[
            B_SH * bq : B_SH * (bq + 1), :, T_SH * rq : T_SH * (rq + 1), :
        ] = results[r]["out"]
    return full


def kernel(h, e, AA_mask, GCW, GCB):
    h = np.asarray(h)
    AA_mask = np.asarray(AA_mask)
    GCW = np.asarray(GCW)
    GCB = np.asarray(GCB)

    compact = _is_tiled(AA_mask)
    key = "compact" if compact else "full"
    if key not in _cached:
        _cached[key] = _build_compact() if compact else _build()
    nc = _cached[key]

    in_maps = _make_in_maps(h, AA_mask, GCW, GCB, compact)
    res = run_bass_kernel_spmd(nc, in_maps, core_ids=list(range(8)))
    return _assemble(res.results)


# revision 20
# speedup vs baseline: 1.1776x; 1.1776x over previous
"""GCNFast Trainium2 kernel.

out[b] = relu(A @ x_b + GCB),  A = relu(AA_mask * GCW)  [4096, 4096]
x_b = transpose(h[b]) reshaped [Nt*Nc, d_h];  out reshaped to [bs, Ns, Nt, d_h].

Sharding over 8 cores: 4-way row-shard of A/GCB (1024 rows each) x 2-way
batch split (8 batches each). Each core computes its [1024, 4096] slice of A
on-chip (DVE mul -> PE transpose to k-major -> ACT relu), keeps the bf16
activations X [4096, 8*128] resident in SBUF, and accumulates 64 bf16
matmuls per m-tile into PSUM, with a DVE bias-add + ACT relu epilogue.

Index conventions inside a core (both are pure permutations absorbed by the
on-chip transpose stage, chosen so every DMA access pattern collapses to
<=3 dims with a contiguous partition merge):
 - contraction k' = c*Nt + t  (c-major), so h's (c t) merges contiguously;
 - output row m' = s*Tsh + t  (s-major), so out's (s t) merges contiguously.
"""

from contextlib import ExitStack

import numpy as np

import concourse.mybir as mybir
import concourse.tile as tile
from concourse import bacc, masks
from concourse.bass_utils import run_bass_kernel_spmd

# Problem constants (hardcoded per harness contract).
NC_, NS, NT, DH, BS = 64, 64, 64, 128, 16
K = NC_ * NT          # 4096 contraction dim
M = NS * NT           # 4096 output rows
P_ROW, P_BATCH = 4, 2  # 4-way row shard x 2-way batch shard = 8 cores
M_SH = M // P_ROW     # 1024 rows per core
B_SH = BS // P_BATCH  # 8 batches per core
NFREE = B_SH * DH     # 1024 = moving free dim (b, d)
KT = K // 128         # 32 k-tiles
MT = M_SH // 128      # 8 m-tiles per core
T_SH = M_SH // NS     # 16 t-values per core
S_PT = 128 // T_SH    # 8 s-values per m'-tile

F32 = mybir.dt.float32
BF16 = mybir.dt.bfloat16

_cached = {}


def _build():
    nc = bacc.Bacc(
        "TRN2",
        target_bir_lowering=False,
        debug=False,
        enable_asserts=False,
        num_devices=8,
    )

    gcw = nc.dram_tensor("gcw", [M_SH, K], F32, kind="ExternalInput").ap()
    aa = nc.dram_tensor("aa", [M_SH, K], F32, kind="ExternalInput").ap()
    gcb = nc.dram_tensor("gcb", [M_SH, DH], F32, kind="ExternalInput").ap()
    h = nc.dram_tensor("h", [B_SH, NC_, NT, DH], F32, kind="ExternalInput").ap()
    out = nc.dram_tensor("out", [B_SH, NS, T_SH, DH], F32, kind="ExternalOutput").ap()

    # row-permuted views: m' = s*T_SH + t  (s-major)
    gcw_p = gcw.rearrange("(t s) k -> s t k", t=T_SH)
    aa_p = aa.rearrange("(t s) k -> s t k", t=T_SH)
    gcb_p = gcb.rearrange("(t s) d -> s t d", t=T_SH)

    with tile.TileContext(nc) as tc:
        with ExitStack() as ctx:
            ident_pool = ctx.enter_context(tc.tile_pool(name="ident", bufs=1))
            x_pool = ctx.enter_context(tc.tile_pool(name="x", bufs=KT))
            gw_pool = ctx.enter_context(tc.tile_pool(name="gw", bufs=4))
            aa_pool = ctx.enter_context(tc.tile_pool(name="aam", bufs=4))
            am_pool = ctx.enter_context(tc.tile_pool(name="am", bufs=2))
            at_pool = ctx.enter_context(tc.tile_pool(name="at", bufs=2))
            gcb_pool = ctx.enter_context(tc.tile_pool(name="gcb", bufs=MT))
            out_pool = ctx.enter_context(tc.tile_pool(name="out", bufs=2))
            ptr_pool = ctx.enter_context(
                tc.tile_pool(name="ptr", bufs=2, space="PSUM")
            )
            pmm_pool = ctx.enter_context(
                tc.tile_pool(name="pmm", bufs=2, space="PSUM")
            )

            ident = ident_pool.tile([128, 128], BF16)
            masks.make_identity(nc, ident[:])

            # Interleave the A-stream prefetch (per-m-tile critical path
            # feeder) with the resident X tiles so neither starves: queue
            # order on the SWDGE ring follows program order.
            gw_tiles, aa_tiles, gcb_tiles, x_tiles = [], [], [], []
            for mt in range(MT):
                srows = slice(S_PT * mt, S_PT * (mt + 1))
                gw_t = gw_pool.tile([128, K], BF16)
                nc.gpsimd.dma_start(out=gw_t[:], in_=gcw_p[srows])
                aa_t = aa_pool.tile([128, K], BF16)
                nc.gpsimd.dma_start(out=aa_t[:], in_=aa_p[srows])
                gw_tiles.append(gw_t)
                aa_tiles.append(aa_t)
                # X[k'-tile] = [128 (c,t), 1024 (b,d)], cast f32->bf16 in
                # the SWDGE DMA datapath; 4 per m-tile covers all 32.
                for j in range(4 * mt, 4 * mt + 4):
                    xt = x_pool.tile([128, NFREE], BF16)
                    src = h[:, 2 * j : 2 * j + 2, :, :].rearrange(
                        "b c t d -> (c t) b d"
                    )
                    nc.gpsimd.dma_start(out=xt[:], in_=src)
                    x_tiles.append(xt)
                if mt == 0:
                    for mt2 in range(MT):
                        srows2 = slice(S_PT * mt2, S_PT * (mt2 + 1))
                        gcb_t = gcb_pool.tile([128, DH], F32)
                        nc.sync.dma_start(out=gcb_t[:], in_=gcb_p[srows2])
                        gcb_tiles.append(gcb_t)

            for mt in range(MT):
                gw_t, aa_t = gw_tiles[mt], aa_tiles[mt]
                # masked weights with fused relu: since aa >= 0,
                # relu(gw*aa) == max(gw,0)*aa. The output AP permutes the
                # free dim from t-major k to c-major k' so the transpose and
                # matmul reads stay dense:
                # am_t[m, c*Nt + t] = max(gw[m, t*Nc+c], 0) * aa[m, t*Nc+c].
                am_t = am_pool.tile([128, K], BF16)
                nc.vector.scalar_tensor_tensor(
                    am_t[:].rearrange("m (c t) -> m t c", c=NC_),
                    gw_t[:].rearrange("m (t c) -> m t c", c=NC_),
                    0.0,
                    aa_t[:].rearrange("m (t c) -> m t c", c=NC_),
                    mybir.AluOpType.max,
                    mybir.AluOpType.mult,
                )

                # A^T for this m'-tile: 32 side-by-side [128 k', 128 m'] tiles.
                at_t = at_pool.tile([128, K], BF16)
                for g in range(KT // 8):
                    ptr = ptr_pool.tile([128, 1024], BF16)
                    for j8 in range(8):
                        j = 8 * g + j8
                        nc.tensor.transpose(
                            ptr[:, 128 * j8 : 128 * j8 + 128],
                            am_t[:, 128 * j : 128 * j + 128],
                            ident[:],
                        )
                    dstslice = at_t[:, 1024 * g : 1024 * g + 1024]
                    if g % 2 == 0:
                        nc.scalar.copy(dstslice, ptr[:])
                    else:
                        nc.vector.tensor_copy(dstslice, ptr[:])

                # 32 accumulating matmuls: psum[m', (b,d)] += A^T[k']^T @ X[k']
                pm = pmm_pool.tile([128, NFREE], F32)
                for j in range(KT):
                    for nh in range(NFREE // 512):
                        nc.tensor.matmul(
                            pm[:, 512 * nh : 512 * nh + 512],
                            at_t[:, 128 * j : 128 * j + 128],
                            x_tiles[j][:, 512 * nh : 512 * nh + 512],
                            start=(j == 0),
                            stop=(j == KT - 1),
                        )

                # epilogue: bias add (broadcast over b) + relu, then store
                o_t = out_pool.tile([128, NFREE], F32)
                bias = gcb_tiles[mt][:].unsqueeze(1).broadcast_to(
                    (128, B_SH, DH)
                )
                nc.vector.tensor_add(
                    o_t[:].rearrange("p (b d) -> p b d", b=B_SH),
                    pm[:].rearrange("p (b d) -> p b d", b=B_SH),
                    bias,
                )
                nc.scalar.activation(
                    o_t[:], o_t[:], mybir.ActivationFunctionType.Relu
                )

                srows = slice(S_PT * mt, S_PT * (mt + 1))
                dst = out[:, srows, :, :].rearrange("b s t d -> (s t) b d")
                nc.sync.dma_start(out=dst, in_=o_t[:])

    nc.compile()
    return nc


def _build_compact():
    """Variant for the (expected) tiled AA_mask: mask[m, k] depends only on
    (m % Ns, k % Nc), so each core loads a tiny per-m-tile [128, Nc] mask
    instead of the full 16.8MB shard -- per-core HBM reads drop ~33%.

    Schedule: a "triangle" of the first 3 m-tiles accumulates both batch
    halves against X tiles as they stream in (6 one-bank PSUM accumulators
    + 2 transpose-staging banks = all of PSUM), so the in-order PE stream
    has matmul work throughout the h/gcw stream. The remaining 5 m-tiles
    run as a PE-bound sequential pipeline fed by trailing gcw loads, which
    have large arrival slack by then."""
    nc = bacc.Bacc(
        "TRN2",
        target_bir_lowering=False,
        debug=False,
        enable_asserts=False,
        num_devices=8,
    )

    gcw = nc.dram_tensor("gcw", [M_SH, K], F32, kind="ExternalInput").ap()
    msk = nc.dram_tensor("msk", [128, MT * NC_], F32, kind="ExternalInput").ap()
    gcb = nc.dram_tensor("gcb", [M_SH, DH], F32, kind="ExternalInput").ap()
    h = nc.dram_tensor("h", [B_SH, NC_, NT, DH], F32, kind="ExternalInput").ap()
    out = nc.dram_tensor("out", [B_SH, NS, T_SH, DH], F32, kind="ExternalOutput").ap()

    gcw_p = gcw.rearrange("(t s) k -> s t k", t=T_SH)
    gcb_p = gcb.rearrange("(t s) d -> s t d", t=T_SH)

    NTRI = 3  # m-tiles in the streaming triangle (both batch halves)

    with tile.TileContext(nc) as tc:
        with ExitStack() as ctx:
            ident_pool = ctx.enter_context(tc.tile_pool(name="ident", bufs=1))
            x_pool = ctx.enter_context(tc.tile_pool(name="x", bufs=KT))
            gw_pool = ctx.enter_context(tc.tile_pool(name="gw", bufs=4))
            msk_pool = ctx.enter_context(tc.tile_pool(name="msk", bufs=1))
            am_pool = ctx.enter_context(tc.tile_pool(name="am", bufs=2))
            at_pool = ctx.enter_context(tc.tile_pool(name="at", bufs=4))
            gcb_pool = ctx.enter_context(tc.tile_pool(name="gcb", bufs=MT))
            out_pool = ctx.enter_context(tc.tile_pool(name="out", bufs=4))
            ps_pool = ctx.enter_context(
                tc.tile_pool(name="ps", bufs=8, space="PSUM")
            )

            ident = ident_pool.tile([128, 128], BF16)
            masks.make_identity(nc, ident[:])

            gcb_tiles, gw_tiles, x_tiles, at_tiles = [], [], [], {}
            pms = {}

            msk_all = msk_pool.tile([128, MT * NC_], BF16)
            nc.gpsimd.dma_start(out=msk_all[:], in_=msk)
            msk_tiles = [
                msk_all[:, NC_ * i : NC_ * (i + 1)] for i in range(MT)
            ]

            def emit_gw_dma(mt):
                srows = slice(S_PT * mt, S_PT * (mt + 1))
                gw_t = gw_pool.tile([128, K], BF16, tag="gw", name=f"gw_{mt}")
                nc.gpsimd.dma_start(out=gw_t[:], in_=gcw_p[srows])
                gw_tiles.append(gw_t)

            def emit_x_dmas(r):
                for j in range(4 * r, 4 * r + 4):
                    xt = x_pool.tile([128, NFREE], BF16, tag="x", name=f"x_{j}")
                    src = h[:, 2 * j : 2 * j + 2, :, :].rearrange(
                        "b c t d -> (c t) b d"
                    )
                    nc.gpsimd.dma_start(out=xt[:], in_=src)
                    x_tiles.append(xt)

            def emit_prep(mt):
                am_t = am_pool.tile([128, K], BF16, tag="am", name=f"am_{mt}")
                at_t = at_pool.tile([128, K], BF16, tag="at", name=f"at_{mt}")
                # am[m, c*Nt+t] = max(gw[m, t*Nc+c], 0) * mask[m, c], in two
                # c-halves so transposes start after half the DVE work
                for ch in range(2):
                    cs = slice(NC_ // 2 * ch, NC_ // 2 * (ch + 1))
                    ks = slice(K // 2 * ch, K // 2 * (ch + 1))
                    nc.vector.scalar_tensor_tensor(
                        am_t[:, ks].rearrange("m (c t) -> m t c", c=NC_ // 2),
                        gw_tiles[mt][:].rearrange("m (t c) -> m t c", c=NC_)[
                            :, :, cs
                        ],
                        0.0,
                        msk_tiles[mt][:, cs].unsqueeze(1).broadcast_to(
                            (128, NT, NC_ // 2)
                        ),
                        mybir.AluOpType.max,
                        mybir.AluOpType.mult,
                    )
                    for g in range(2 * ch, 2 * ch + 2):
                        ptr = ps_pool.tile(
                            [128, 1024], BF16, tag="ps", name=f"ptr_{g}"
                        )
                        for j8 in range(8):
                            j = 8 * g + j8
                            nc.tensor.transpose(
                                ptr[:, 128 * j8 : 128 * j8 + 128],
                                am_t[:, 128 * j : 128 * j + 128],
                                ident[:],
                            )
                        dstslice = at_t[:, 1024 * g : 1024 * g + 1024]
                        if g % 2 == 0:
                            nc.scalar.copy(dstslice, ptr[:])
                        else:
                            nc.vector.tensor_copy(dstslice, ptr[:])
                at_tiles[mt] = at_t

            def emit_mms(mt, ks, bh):
                pm = pms[(mt, bh)]
                at_t = at_tiles[mt]
                for k in ks:
                    nc.tensor.matmul(
                        pm[:],
                        at_t[:, 128 * k : 128 * k + 128],
                        x_tiles[k][:, 512 * bh : 512 * bh + 512],
                        start=(k == 0),
                        stop=(k == KT - 1),
                    )

            def emit_epi(mt, bh):
                pm = pms.pop((mt, bh))
                o_t = out_pool.tile([128, 512], F32, tag="out", name=f"o_{mt}_{bh}")
                bias = gcb_tiles[mt][:].unsqueeze(1).broadcast_to(
                    (128, 4, DH)
                )
                nc.vector.tensor_add(
                    o_t[:].rearrange("p (b d) -> p b d", b=4),
                    pm[:].rearrange("p (b d) -> p b d", b=4),
                    bias,
                )
                nc.scalar.activation(
                    o_t[:], o_t[:], mybir.ActivationFunctionType.Relu
                )
                srows = slice(S_PT * mt, S_PT * (mt + 1))
                dst = out[4 * bh : 4 * bh + 4, srows, :, :].rearrange(
                    "b s t d -> (s t) b d"
                )
                nc.sync.dma_start(out=dst, in_=o_t[:])

            def alloc_pm(mt, bh):
                pms[(mt, bh)] = ps_pool.tile(
                    [128, 512], F32, tag="ps", name=f"pm_{mt}_{bh}"
                )

            # ---- DMA + compute emission ----
            # streaming phase: gcw(0..2) early, X windows, triangle MMs
            for r in range(MT):
                if r < NTRI:
                    emit_gw_dma(r)
                emit_x_dmas(r)
                if r == 0:
                    for i in range(MT):
                        srows2 = slice(S_PT * i, S_PT * (i + 1))
                        gcb_t = gcb_pool.tile(
                            [128, DH], F32, tag="gcb", name=f"gcb_{i}"
                        )
                        nc.sync.dma_start(out=gcb_t[:], in_=gcb_p[srows2])
                        gcb_tiles.append(gcb_t)
                if r < NTRI:
                    emit_prep(r)
                for mt in range(min(r, NTRI - 1) + 1):
                    if mt == r:
                        alloc_pm(mt, 0)
                        alloc_pm(mt, 1)
                        ks = range(0, 4 * r + 4)
                    else:
                        ks = range(4 * r, 4 * r + 4)
                    for k in ks:
                        for bh in range(2):
                            emit_mms(mt, [k], bh)

            # trailing gcw loads: needed only as the sequential tail
            # consumes them, well after the X stream completes
            for mt in range(NTRI, MT):
                emit_gw_dma(mt)

            # triangle epilogues, then the PE-bound sequential tail
            for mt in range(NTRI):
                emit_epi(mt, 0)
                emit_epi(mt, 1)
            for mt in range(NTRI, MT):
                emit_prep(mt)
                for bh in range(2):
                    alloc_pm(mt, bh)
                    emit_mms(mt, range(KT), bh)
                    emit_epi(mt, bh)

    nc.compile()
    return nc


def _mask_small(AA_mask):
    """[128, MT*Nc] per-m'-tile mask rows, mt-major along the free dim
    (identical for every core)."""
    A64 = AA_mask[:NS, :NC_]
    ms = np.empty((128, MT * NC_), dtype=np.float32)
    for mt in range(MT):
        for p in range(128):
            s = S_PT * mt + p // T_SH
            ms[p, NC_ * mt : NC_ * (mt + 1)] = A64[s]
    return ms


def _is_tiled(AA_mask):
    A64 = AA_mask[:NS, :NC_]
    return np.array_equal(AA_mask, np.tile(A64, (NT, NT)))


def _make_in_maps(h, AA_mask, GCW, GCB, compact):
    in_maps = []
    ms = _mask_small(AA_mask) if compact else None
    for r in range(8):
        rq, bq = r % P_ROW, r // P_ROW
        rs = slice(M_SH * rq, M_SH * (rq + 1))
        bs_ = slice(B_SH * bq, B_SH * (bq + 1))
        m = {
            "gcw": np.ascontiguousarray(GCW[rs], np.float32),
            "gcb": np.ascontiguousarray(GCB[rs], np.float32),
            "h": np.ascontiguousarray(h[bs_], np.float32),
        }
        if compact:
            m["msk"] = ms
        else:
            m["aa"] = np.ascontiguousarray(AA_mask[rs], np.float32)
        in_maps.append(m)
    return in_maps


def _assemble(results):
    full = np.empty((BS, NS, NT, DH), dtype=np.float32)
    for r in range(8):
        rq, bq = r % P_ROW, r // P_ROW
        full[
            B_SH * bq : B_SH * (bq + 1), :, T_SH * rq : T_SH * (rq + 1), :
        ] = results[r]["out"]
    return full


def kernel(h, e, AA_mask, GCW, GCB):
    h = np.asarray(h)
    AA_mask = np.asarray(AA_mask)
    GCW = np.asarray(GCW)
    GCB = np.asarray(GCB)

    compact = _is_tiled(AA_mask)
    key = "compact" if compact else "full"
    if key not in _cached:
        _cached[key] = _build_compact() if compact else _build()
    nc = _cached[key]

    in_maps = _make_in_maps(h, AA_mask, GCW, GCB, compact)
    res = run_bass_kernel_spmd(nc, in_maps, core_ids=list(range(8)))
    return _assemble(res.results)


# revision 23
# speedup vs baseline: 1.2551x; 1.0659x over previous
"""GCNFast Trainium2 kernel.

out[b] = relu(A @ x_b + GCB),  A = relu(AA_mask * GCW)  [4096, 4096]
x_b = transpose(h[b]) reshaped [Nt*Nc, d_h];  out reshaped to [bs, Ns, Nt, d_h].

Sharding over 8 cores: 4-way row-shard of A/GCB (1024 rows each) x 2-way
batch split (8 batches each). Each core computes its slice of A on-chip
(DVE masked-relu mul -> PE transpose to contraction-major), keeps the bf16
activations X [4096, 8*128] resident in SBUF, and accumulates bf16 matmuls
into PSUM with a DVE bias-add + ACT relu epilogue. bf16 operands keep the
relative error ~2e-3 (inputs quantized once; accumulation in fp32 PSUM).

Two compiled variants, selected at runtime:
 - compact: AA_mask is tile(AA, (Nt, Nt)) (what setup_inputs produces), so
   only a [128, Nc] per-m-tile mask is loaded and broadcast along t. That
   drops per-core HBM reads from ~50MB to ~34MB. Scheduling: a "triangle"
   of the first 4 m-tiles accumulates both batch halves against X tiles as
   they stream in (8 one-bank PSUM accumulators; the 2 transpose-staging
   banks are handed over exactly when the 4th pair allocates), then the
   remaining 4 m-tiles run as a PE-bound sequential pipeline fed by
   trailing gcw loads.
 - full: general AA_mask fallback (full mask shard streamed, simple
   m-tile pipeline).

Index conventions inside a core (both are pure permutations absorbed by the
on-chip transpose stage, chosen so every DMA access pattern collapses to
<=3 dims with a contiguous partition merge):
 - contraction k' = c*Nt + t  (c-major), so h's (c t) merges contiguously;
 - output row m' = s*Tsh + t  (s-major), so out's (s t) merges contiguously.
"""

from contextlib import ExitStack

import numpy as np

import concourse.mybir as mybir
import concourse.tile as tile
from concourse import bacc, masks
from concourse.bass_utils import run_bass_kernel_spmd

# Problem constants (hardcoded per harness contract).
NC_, NS, NT, DH, BS = 64, 64, 64, 128, 16
K = NC_ * NT          # 4096 contraction dim
M = NS * NT           # 4096 output rows
P_ROW, P_BATCH = 4, 2  # 4-way row shard x 2-way batch shard = 8 cores
M_SH = M // P_ROW     # 1024 rows per core
B_SH = BS // P_BATCH  # 8 batches per core
NFREE = B_SH * DH     # 1024 = moving free dim (b, d)
KT = K // 128         # 32 k-tiles
MT = M_SH // 128      # 8 m-tiles per core
T_SH = M_SH // NS     # 16 t-values per core
S_PT = 128 // T_SH    # 8 s-values per m'-tile

F32 = mybir.dt.float32
BF16 = mybir.dt.bfloat16

_cached = {}


def _build():
    nc = bacc.Bacc(
        "TRN2",
        target_bir_lowering=False,
        debug=False,
        enable_asserts=False,
        num_devices=8,
    )

    gcw = nc.dram_tensor("gcw", [M_SH, K], F32, kind="ExternalInput").ap()
    aa = nc.dram_tensor("aa", [M_SH, K], F32, kind="ExternalInput").ap()
    gcb = nc.dram_tensor("gcb", [M_SH, DH], F32, kind="ExternalInput").ap()
    h = nc.dram_tensor("h", [B_SH, NC_, NT, DH], F32, kind="ExternalInput").ap()
    out = nc.dram_tensor("out", [B_SH, NS, T_SH, DH], F32, kind="ExternalOutput").ap()

    # row-permuted views: m' = s*T_SH + t  (s-major)
    gcw_p = gcw.rearrange("(t s) k -> s t k", t=T_SH)
    aa_p = aa.rearrange("(t s) k -> s t k", t=T_SH)
    gcb_p = gcb.rearrange("(t s) d -> s t d", t=T_SH)

    with tile.TileContext(nc) as tc:
        with ExitStack() as ctx:
            ident_pool = ctx.enter_context(tc.tile_pool(name="ident", bufs=1))
            x_pool = ctx.enter_context(tc.tile_pool(name="x", bufs=KT))
            gw_pool = ctx.enter_context(tc.tile_pool(name="gw", bufs=4))
            aa_pool = ctx.enter_context(tc.tile_pool(name="aam", bufs=4))
            am_pool = ctx.enter_context(tc.tile_pool(name="am", bufs=2))
            at_pool = ctx.enter_context(tc.tile_pool(name="at", bufs=2))
            gcb_pool = ctx.enter_context(tc.tile_pool(name="gcb", bufs=MT))
            out_pool = ctx.enter_context(tc.tile_pool(name="out", bufs=2))
            ptr_pool = ctx.enter_context(
                tc.tile_pool(name="ptr", bufs=2, space="PSUM")
            )
            pmm_pool = ctx.enter_context(
                tc.tile_pool(name="pmm", bufs=2, space="PSUM")
            )

            ident = ident_pool.tile([128, 128], BF16)
            masks.make_identity(nc, ident[:])

            # Interleave the A-stream prefetch (per-m-tile critical path
            # feeder) with the resident X tiles so neither starves: queue
            # order on the SWDGE ring follows program order.
            gw_tiles, aa_tiles, gcb_tiles, x_tiles = [], [], [], []
            for mt in range(MT):
                srows = slice(S_PT * mt, S_PT * (mt + 1))
                gw_t = gw_pool.tile([128, K], BF16)
                nc.gpsimd.dma_start(out=gw_t[:], in_=gcw_p[srows])
                aa_t = aa_pool.tile([128, K], BF16)
                nc.gpsimd.dma_start(out=aa_t[:], in_=aa_p[srows])
                gw_tiles.append(gw_t)
                aa_tiles.append(aa_t)
                # X[k'-tile] = [128 (c,t), 1024 (b,d)], cast f32->bf16 in
                # the SWDGE DMA datapath; 4 per m-tile covers all 32.
                for j in range(4 * mt, 4 * mt + 4):
                    xt = x_pool.tile([128, NFREE], BF16)
                    src = h[:, 2 * j : 2 * j + 2, :, :].rearrange(
                        "b c t d -> (c t) b d"
                    )
                    nc.gpsimd.dma_start(out=xt[:], in_=src)
                    x_tiles.append(xt)
                if mt == 0:
                    for mt2 in range(MT):
                        srows2 = slice(S_PT * mt2, S_PT * (mt2 + 1))
                        gcb_t = gcb_pool.tile([128, DH], F32)
                        nc.sync.dma_start(out=gcb_t[:], in_=gcb_p[srows2])
                        gcb_tiles.append(gcb_t)

            for mt in range(MT):
                gw_t, aa_t = gw_tiles[mt], aa_tiles[mt]
                # masked weights with fused relu: since aa >= 0,
                # relu(gw*aa) == max(gw,0)*aa. The output AP permutes the
                # free dim from t-major k to c-major k' so the transpose and
                # matmul reads stay dense:
                # am_t[m, c*Nt + t] = max(gw[m, t*Nc+c], 0) * aa[m, t*Nc+c].
                am_t = am_pool.tile([128, K], BF16)
                nc.vector.scalar_tensor_tensor(
                    am_t[:].rearrange("m (c t) -> m t c", c=NC_),
                    gw_t[:].rearrange("m (t c) -> m t c", c=NC_),
                    0.0,
                    aa_t[:].rearrange("m (t c) -> m t c", c=NC_),
                    mybir.AluOpType.max,
                    mybir.AluOpType.mult,
                )

                # A^T for this m'-tile: 32 side-by-side [128 k', 128 m'] tiles.
                at_t = at_pool.tile([128, K], BF16)
                for g in range(KT // 8):
                    ptr = ptr_pool.tile([128, 1024], BF16)
                    for j8 in range(8):
                        j = 8 * g + j8
                        nc.tensor.transpose(
                            ptr[:, 128 * j8 : 128 * j8 + 128],
                            am_t[:, 128 * j : 128 * j + 128],
                            ident[:],
                        )
                    dstslice = at_t[:, 1024 * g : 1024 * g + 1024]
                    if g % 2 == 0:
                        nc.scalar.copy(dstslice, ptr[:])
                    else:
                        nc.vector.tensor_copy(dstslice, ptr[:])

                # 32 accumulating matmuls: psum[m', (b,d)] += A^T[k']^T @ X[k']
                pm = pmm_pool.tile([128, NFREE], F32)
                for j in range(KT):
                    for nh in range(NFREE // 512):
                        nc.tensor.matmul(
                            pm[:, 512 * nh : 512 * nh + 512],
                            at_t[:, 128 * j : 128 * j + 128],
                            x_tiles[j][:, 512 * nh : 512 * nh + 512],
                            start=(j == 0),
                            stop=(j == KT - 1),
                        )

                # epilogue: bias add (broadcast over b) + relu, then store
                o_t = out_pool.tile([128, NFREE], F32)
                bias = gcb_tiles[mt][:].unsqueeze(1).broadcast_to(
                    (128, B_SH, DH)
                )
                nc.vector.tensor_add(
                    o_t[:].rearrange("p (b d) -> p b d", b=B_SH),
                    pm[:].rearrange("p (b d) -> p b d", b=B_SH),
                    bias,
                )
                nc.scalar.activation(
                    o_t[:], o_t[:], mybir.ActivationFunctionType.Relu
                )

                srows = slice(S_PT * mt, S_PT * (mt + 1))
                dst = out[:, srows, :, :].rearrange("b s t d -> (s t) b d")
                nc.sync.dma_start(out=dst, in_=o_t[:])

    nc.compile()
    return nc


def _build_compact():
    """Variant for the (expected) tiled AA_mask: mask[m, k] depends only on
    (m % Ns, k % Nc), so each core loads a tiny per-m-tile [128, Nc] mask
    instead of the full 16.8MB shard -- per-core HBM reads drop ~33%.

    Schedule: a "triangle" of the first 3 m-tiles accumulates both batch
    halves against X tiles as they stream in (6 one-bank PSUM accumulators
    + 2 transpose-staging banks = all of PSUM), so the in-order PE stream
    has matmul work throughout the h/gcw stream. The remaining 5 m-tiles
    run as a PE-bound sequential pipeline fed by trailing gcw loads, which
    have large arrival slack by then."""
    nc = bacc.Bacc(
        "TRN2",
        target_bir_lowering=False,
        debug=False,
        enable_asserts=False,
        num_devices=8,
    )

    gcw = nc.dram_tensor("gcw", [M_SH, K], F32, kind="ExternalInput").ap()
    msk = nc.dram_tensor("msk", [128, MT * NC_], F32, kind="ExternalInput").ap()
    gcb = nc.dram_tensor("gcb", [M_SH, DH], F32, kind="ExternalInput").ap()
    h = nc.dram_tensor("h", [B_SH, NC_, NT, DH], F32, kind="ExternalInput").ap()
    out = nc.dram_tensor("out", [B_SH, NS, T_SH, DH], F32, kind="ExternalOutput").ap()

    gcw_p = gcw.rearrange("(t s) k -> s t k", t=T_SH)
    gcb_p = gcb.rearrange("(t s) d -> s t d", t=T_SH)

    NTRI = 4  # m-tiles in the streaming triangle (both batch halves)

    with tile.TileContext(nc) as tc:
        with ExitStack() as ctx:
            ident_pool = ctx.enter_context(tc.tile_pool(name="ident", bufs=1))
            x_pool = ctx.enter_context(tc.tile_pool(name="x", bufs=KT))
            gw_pool = ctx.enter_context(tc.tile_pool(name="gw", bufs=4))
            msk_pool = ctx.enter_context(tc.tile_pool(name="msk", bufs=1))
            am_pool = ctx.enter_context(tc.tile_pool(name="am", bufs=2))
            at_pool = ctx.enter_context(tc.tile_pool(name="at", bufs=5))
            gcb_pool = ctx.enter_context(tc.tile_pool(name="gcb", bufs=MT))
            out_pool = ctx.enter_context(tc.tile_pool(name="out", bufs=4))
            ps_pool = ctx.enter_context(
                tc.tile_pool(name="ps", bufs=8, space="PSUM")
            )

            ident = ident_pool.tile([128, 128], BF16)
            masks.make_identity(nc, ident[:])

            gcb_tiles, gw_tiles, x_tiles, at_tiles = [], [], [], {}
            pms = {}

            msk_f32 = msk_pool.tile([128, MT * NC_], F32)
            nc.sync.dma_start(out=msk_f32[:], in_=msk)
            msk_all = msk_pool.tile([128, MT * NC_], BF16)
            nc.vector.tensor_copy(msk_all[:], msk_f32[:])
            msk_tiles = [
                msk_all[:, NC_ * i : NC_ * (i + 1)] for i in range(MT)
            ]

            def emit_gw_dma(mt):
                srows = slice(S_PT * mt, S_PT * (mt + 1))
                gw_t = gw_pool.tile([128, K], BF16, tag="gw", name=f"gw_{mt}")
                nc.gpsimd.dma_start(out=gw_t[:], in_=gcw_p[srows])
                gw_tiles.append(gw_t)

            def emit_x_dmas(r):
                for j in range(4 * r, 4 * r + 4):
                    xt = x_pool.tile([128, NFREE], BF16, tag="x", name=f"x_{j}")
                    src = h[:, 2 * j : 2 * j + 2, :, :].rearrange(
                        "b c t d -> (c t) b d"
                    )
                    nc.gpsimd.dma_start(out=xt[:], in_=src)
                    x_tiles.append(xt)

            def emit_prep(mt):
                am_t = am_pool.tile([128, K], BF16, tag="am", name=f"am_{mt}")
                at_t = at_pool.tile([128, K], BF16, tag="at", name=f"at_{mt}")
                # am[m, c*Nt+t] = max(gw[m, t*Nc+c], 0) * mask[m, c], in two
                # c-halves so transposes start after half the DVE work
                for ch in range(2):
                    cs = slice(NC_ // 2 * ch, NC_ // 2 * (ch + 1))
                    ks = slice(K // 2 * ch, K // 2 * (ch + 1))
                    nc.vector.scalar_tensor_tensor(
                        am_t[:, ks].rearrange("m (c t) -> m t c", c=NC_ // 2),
                        gw_tiles[mt][:].rearrange("m (t c) -> m t c", c=NC_)[
                            :, :, cs
                        ],
                        0.0,
                        msk_tiles[mt][:, cs].unsqueeze(1).broadcast_to(
                            (128, NT, NC_ // 2)
                        ),
                        mybir.AluOpType.max,
                        mybir.AluOpType.mult,
                    )
                    for g in range(2 * ch, 2 * ch + 2):
                        ptr = ps_pool.tile(
                            [128, 1024], BF16, tag="ps", name=f"ptr_{g}"
                        )
                        for j8 in range(8):
                            j = 8 * g + j8
                            nc.tensor.transpose(
                                ptr[:, 128 * j8 : 128 * j8 + 128],
                                am_t[:, 128 * j : 128 * j + 128],
                                ident[:],
                            )
                        dstslice = at_t[:, 1024 * g : 1024 * g + 1024]
                        if g % 2 == 0:
                            nc.scalar.copy(dstslice, ptr[:])
                        else:
                            nc.vector.tensor_copy(dstslice, ptr[:])
                at_tiles[mt] = at_t

            def emit_mms(mt, ks, bh):
                pm = pms[(mt, bh)]
                at_t = at_tiles[mt]
                for k in ks:
                    nc.tensor.matmul(
                        pm[:],
                        at_t[:, 128 * k : 128 * k + 128],
                        x_tiles[k][:, 512 * bh : 512 * bh + 512],
                        start=(k == 0),
                        stop=(k == KT - 1),
                    )

            def emit_epi(mt, bh):
                pm = pms.pop((mt, bh))
                o_t = out_pool.tile([128, 512], F32, tag="out", name=f"o_{mt}_{bh}")
                bias = gcb_tiles[mt][:].unsqueeze(1).broadcast_to(
                    (128, 4, DH)
                )
                nc.vector.tensor_add(
                    o_t[:].rearrange("p (b d) -> p b d", b=4),
                    pm[:].rearrange("p (b d) -> p b d", b=4),
                    bias,
                )
                nc.scalar.activation(
                    o_t[:], o_t[:], mybir.ActivationFunctionType.Relu
                )
                srows = slice(S_PT * mt, S_PT * (mt + 1))
                dst = out[4 * bh : 4 * bh + 4, srows, :, :].rearrange(
                    "b s t d -> (s t) b d"
                )
                nc.sync.dma_start(out=dst, in_=o_t[:])

            def alloc_pm(mt, bh):
                pms[(mt, bh)] = ps_pool.tile(
                    [128, 512], F32, tag="ps", name=f"pm_{mt}_{bh}"
                )

            # ---- DMA + compute emission ----
            # streaming phase: gcw(0..2) early, X windows, triangle MMs
            for r in range(MT):
                if r < NTRI:
                    emit_gw_dma(r)
                emit_x_dmas(r)
                if r == 0:
                    for i in range(MT):
                        srows2 = slice(S_PT * i, S_PT * (i + 1))
                        gcb_t = gcb_pool.tile(
                            [128, DH], F32, tag="gcb", name=f"gcb_{i}"
                        )
                        nc.sync.dma_start(out=gcb_t[:], in_=gcb_p[srows2])
                        gcb_tiles.append(gcb_t)
                if r < NTRI:
                    emit_prep(r)
                for mt in range(min(r, NTRI - 1) + 1):
                    if mt == r:
                        alloc_pm(mt, 0)
                        alloc_pm(mt, 1)
                        ks = range(0, 4 * r + 4)
                    else:
                        ks = range(4 * r, 4 * r + 4)
                    for k in ks:
                        for bh in range(2):
                            emit_mms(mt, [k], bh)

            # trailing gcw loads: needed only as the sequential tail
            # consumes them, well after the X stream completes
            for mt in range(NTRI, MT):
                emit_gw_dma(mt)

            # triangle epilogues, then the PE-bound sequential tail
            for mt in range(NTRI):
                emit_epi(mt, 0)
                emit_epi(mt, 1)
            for mt in range(NTRI, MT):
                emit_prep(mt)
                for bh in range(2):
                    alloc_pm(mt, bh)
                    emit_mms(mt, range(KT), bh)
                    emit_epi(mt, bh)

    nc.compile()
    return nc


def _mask_small(AA_mask):
    """[128, MT*Nc] per-m'-tile mask rows, mt-major along the free dim
    (identical for every core)."""
    A64 = AA_mask[:NS, :NC_]
    ms = np.empty((128, MT * NC_), dtype=np.float32)
    for mt in range(MT):
        for p in range(128):
            s = S_PT * mt + p // T_SH
            ms[p, NC_ * mt : NC_ * (mt + 1)] = A64[s]
    return ms


def _is_tiled(AA_mask):
    A64 = AA_mask[:NS, :NC_]
    return np.array_equal(AA_mask, np.tile(A64, (NT, NT)))


def _make_in_maps(h, AA_mask, GCW, GCB, compact):
    in_maps = []
    ms = _mask_small(AA_mask) if compact else None
    for r in range(8):
        rq, bq = r % P_ROW, r // P_ROW
        rs = slice(M_SH * rq, M_SH * (rq + 1))
        bs_ = slice(B_SH * bq, B_SH * (bq + 1))
        m = {
            "gcw": np.ascontiguousarray(GCW[rs], np.float32),
            "gcb": np.ascontiguousarray(GCB[rs], np.float32),
            "h": np.ascontiguousarray(h[bs_], np.float32),
        }
        if compact:
            m["msk"] = ms
        else:
            m["aa"] = np.ascontiguousarray(AA_mask[rs], np.float32)
        in_maps.append(m)
    return in_maps


def _assemble(results):
    full = np.empty((BS, NS, NT, DH), dtype=np.float32)
    for r in range(8):
        rq, bq = r % P_ROW, r // P_ROW
        full[
            B_SH * bq : B_SH * (bq + 1), :, T_SH * rq : T_SH * (rq + 1), :
        ] = results[r]["out"]
    return full


def kernel(h, e, AA_mask, GCW, GCB):
    h = np.asarray(h)
    AA_mask = np.asarray(AA_mask)
    GCW = np.asarray(GCW)
    GCB = np.asarray(GCB)

    compact = _is_tiled(AA_mask)
    key = "compact" if compact else "full"
    if key not in _cached:
        _cached[key] = _build_compact() if compact else _build()
    nc = _cached[key]

    in_maps = _make_in_maps(h, AA_mask, GCW, GCB, compact)
    res = run_bass_kernel_spmd(nc, in_maps, core_ids=list(range(8)))
    return _assemble(res.results)
